# revision 18
# baseline (speedup 1.0000x reference)
"""PoPE attention kernel for Trainium2, sharded over 8 NeuronCores by heads.

Problem: B=1, S=2048, DIM=1024, H=16 heads, D=64.
  q/k/v = x @ w{q,k,v}^T ; PoPE embed (softplus magnitude x cos/sin phase);
  scores = q_emb @ k_emb^T / sqrt(D); softmax; out = attn @ v; y = out @ wo^T.

Sharding: 2 heads per core. Each core computes its heads' projections,
attention, and a partial output projection (its 128 channels of wo);
host sums the 8 partial y's (f32) - no on-chip collectives.

Key scheduling facts (from ntff traces):
  - ACT paces attention at ~1.2us per [128,1024] exp chunk (64 chunks).
  - PE streams at 0.417 ns/col once warm; total ~84us of matmul columns.
  - Input DMA is ~350 GB/s per core; every input byte delays the start.
Optimizations vs the first working version:
  - Activation tables pinned to natural_log_exp_and_others (exp+ln+copy in
    one set) by masking exp/ln out of the smaller sets before bacc's
    table-load pass: removes 2x 1.5us ACT_TABLE_LOAD switches.
  - trig tables: only cos/sin(freqs) [64,2,S] is DMA'd (heads share it);
    the per-head key phases cos/sin(freqs+bias) are built on DVE via angle
    addition from tiny per-head bias cos/sin - saves 1.5 MB of input DMA.
  - v-projection PSUM evictions go to GpSimd so DVE can start the embed
    multiplies (which gate the first scores matmul) immediately after
    softplus.
  - softmax 1/rowsum: rowsum row is evicted first (bf16), spread over 128
    partitions by DMA, reciprocal on DVE, DMA back, gpsimd broadcast.
    The attn@v accumulator is evicted in bf16 and the normalize multiply
    runs in DVE 2x mode.
  - tail: the last (head, superblock) normalize uses ACT ln+exp (same
    table, ACT is idle in the tail) directly from PSUM instead of the DMA
    round-trip, split in column halves, and the final output projection
    chases it in [128,512] pieces with evictions spread over DVE/ACT/GpSimd.
"""
import math

import numpy as np
import ml_dtypes

import concourse.bacc as bacc
import concourse.mybir as mybir
from concourse import tile
from concourse.bass_utils import run_bass_kernel_spmd

BF16 = ml_dtypes.bfloat16
S, DIM, H, D = 2048, 1024, 16, 64
NCORES = 8
HPC = H // NCORES          # heads per core = 2
ED = 2 * D                 # embedding width per head = 128
KI = DIM // 128            # contraction chunks for projections = 8
KC = S // 128              # key-token chunks = 16
QC = S // 512              # query free-dim chunks of 512 = 4
OC = DIM // 128            # output-channel chunks = 8
QH = 1024                  # query superblock width

_compiled_nc = None
_act_tables_patched = False


def _patch_act_tables():
    """Make bacc's activation-table pass pick natural_log_exp_and_others for
    both Exp and Ln (they otherwise land in two different sets and every
    Exp<->Ln transition costs a ~1.5us ACT_TABLE_LOAD). Masking exp/ln out
    of the smaller sets preserves set indices (act_func_set_id is
    positional) while forcing the combined set."""
    global _act_tables_patched
    if _act_tables_patched:
        return
    _act_tables_patched = True
    orig = bacc.get_activation_tables

    def patched(arch):
        tabs = dict(orig(arch))
        AF = mybir.ActivationFunctionType
        combined = None
        for name, fns in tabs.items():
            if AF.Exp in fns and AF.Ln in fns:
                combined = name
                break
        if combined is None:
            return tabs
        for name, fns in tabs.items():
            if name != combined:
                tabs[name] = fns - {AF.Exp, AF.Ln}
        return tabs

    bacc.get_activation_tables = patched


def _build_body(nc, tc, persist, ps_pool, out_pool, xt_pool, exp_pool, ext):
    dt = mybir.dt
    AF = mybir.ActivationFunctionType
    ALU = mybir.AluOpType
    xt_ext, wqk_ext, wv_ext, tq_ext, btr_ext, wo_ext, y_ext = ext

    # ---- HAM warmup: dummy matmuls on junk data while the input DMAs run,
    # so the PE clock-gate reaches 2.4 GHz before the real matmuls start ----
    warm_sb = persist.tile([128, 512], dt.bfloat16)
    nc.gpsimd.memset(warm_sb[:], 0.0)
    warm_ps = ps_pool.tile([128, 512], dt.float32, name="warm_ps", tag="scA")
    for i in range(16):
        nc.tensor.matmul(warm_ps[:], warm_sb[:, 0:128], warm_sb[:],
                         start=(i == 0), stop=(i == 15))

    # ---- phase A: input DMAs (wqk and xt first so matmuls start early) ----
    wqk_sb = persist.tile([128, 2, KI, ED], dt.bfloat16)
    nc.sync.dma_start(wqk_sb[:], wqk_ext[:])
    xt = xt_pool.tile([128, KI, S], dt.bfloat16)
    for ki in range(KI):
        nc.sync.dma_start(xt[:, ki, :], xt_ext[:, ki, :])
    tq_sb = persist.tile([128, 2, S], dt.bfloat16)
    nc.sync.dma_start(tq_sb[:], tq_ext[:])
    btr_sb = persist.tile([128, 2], dt.float32)
    nc.sync.dma_start(btr_sb[:], btr_ext[:])
    wv_sb = persist.tile([128, KI, ED], dt.bfloat16)
    nc.sync.dma_start(wv_sb[:], wv_ext[:])
    wo_sb = persist.tile([128, DIM], dt.bfloat16)
    nc.sync.dma_start(wo_sb[:], wo_ext[:])

    # key-phase trig tables built on DVE (idle during the input DMA):
    # ck = cq*cos(b) - sq*sin(b); sk = sq*cos(b) + cq*sin(b). btr is
    # head-major per partition so both heads go in one full-width op.
    tk_sb = persist.tile([128, 2, S], dt.bfloat16)
    # scratch shares the softplus tmp tag (trig_k finishes before softplus)
    tk_tmp = xt_pool.tile([128, S], dt.bfloat16, name="tktmp", tag="sp")
    cq, sq = tq_sb[:, 0, :], tq_sb[:, 1, :]
    cb = btr_sb[:, 0:1]
    sb = btr_sb[:, 1:2]
    nc.vector.tensor_scalar_mul(tk_tmp[:], sq, sb)
    nc.vector.scalar_tensor_tensor(
        tk_sb[:, 0, :], cq, cb, tk_tmp[:], ALU.mult, ALU.subtract)
    nc.vector.tensor_scalar_mul(tk_tmp[:], cq, sb)
    nc.vector.scalar_tensor_tensor(
        tk_sb[:, 1, :], sq, cb, tk_tmp[:], ALU.mult, ALU.add)

    # v with a ones column appended per (head, key chunk)
    v_sb = persist.tile([128, HPC, KC, D + 1], dt.bfloat16)
    nc.gpsimd.memset(v_sb[:, 0, :, D], 1.0)
    nc.gpsimd.memset(v_sb[:, 1, :, D], 1.0)
    emb_q = [persist.tile([128, S], dt.bfloat16, name=f"embq{h}", tag=f"embq{h}")
             for h in range(HPC)]
    emb_k = [persist.tile([128, S], dt.bfloat16, name=f"embk{h}", tag=f"embk{h}")
             for h in range(HPC)]
    outT = persist.tile([128, S], dt.bfloat16)

    # PSUM layout: four 2-bank tags. Scores ping-pong on scA/scB while the
    # two attn@v accumulators sit on avA/avB; the projections and the output
    # projection reuse the same four tags.
    # ---- phase B: q/k projections (ki-outer so MMs start after the first
    # xt chunk lands), clustered softplus ----
    psm = {}
    for p in range(2):  # 0=q, 1=k
        for lo in range(2):
            t = ps_pool.tile([128, QH], dt.float32, name=f"psm{p}_{lo}",
                             tag=("scA", "avA", "scB", "avB")[2 * p + lo])
            psm[(p, lo)] = t
        for ki in range(KI):
            for qc in range(QC):
                nc.tensor.matmul(
                    psm[(p, qc // 2)][:, (qc % 2) * 512:(qc % 2) * 512 + 512],
                    wqk_sb[:, p, ki, :],
                    xt[:, ki, qc * 512:(qc + 1) * 512],
                    start=(ki == 0), stop=(ki == KI - 1),
                )
    # softplus(x) = ln(1 + e^x); exp and ln share one activation table
    # (pinned by _patch_act_tables) so no table switches occur.
    tmp = xt_pool.tile([128, 2, S], dt.float32, name="sp", tag="sp")
    mag = xt_pool.tile([128, 2, S], dt.bfloat16, name="mag", tag="mag")
    qk_mag = [mag[:, 0, :], mag[:, 1, :]]
    for p in range(2):
        for lo in range(2):
            nc.scalar.activation(tmp[:, p, lo * QH:(lo + 1) * QH],
                                 psm[(p, lo)][:], AF.Exp)
    nc.scalar.activation(mag[:, :, 0:QH], tmp[:, :, 0:QH], AF.Ln, bias=1.0)
    nc.scalar.activation(mag[:, :, QH:S], tmp[:, :, QH:S], AF.Ln, bias=1.0)

    # ---- phase C: v projection (token-major directly). Only the scA/scB
    # groups' evictions precede the first embeds on the DVE queue; the
    # avA/avB groups evict after the first scores are unblocked. ----
    def v_group_mm(g, tag):
        psv = ps_pool.tile([128, 4, 128], dt.float32, name=f"psv{g}", tag=tag)
        for sub in range(4):
            t = 4 * g + sub
            for ki in range(KI):
                nc.tensor.matmul(
                    psv[:, sub, :],
                    xt[:, ki, t * 128:(t + 1) * 128],
                    wv_sb[:, ki, :],
                    start=(ki == 0), stop=(ki == KI - 1),
                )
        return psv

    def v_group_evict(g, psv):
        for h in range(HPC):
            for sub in range(4):
                t = 4 * g + sub
                nc.vector.tensor_copy(
                    v_sb[:, h, t, 0:D], psv[:, sub, 64 * h:64 * h + 64])

    psvs = [v_group_mm(g, tag)
            for g, tag in enumerate(("scA", "scB", "avA", "avB"))]
    v_group_evict(0, psvs[0])
    v_group_evict(1, psvs[1])

    # embeds on DVE (bf16 SBUF 2x mode), head 0's low half first so its
    # first score chunks can start sooner
    def emb_mul(lo, h):
        c = slice(lo * QH, (lo + 1) * QH)
        r = slice(64 * h, 64 * h + 64)
        for t in range(2):  # 0=cos part, 1=sin part
            e = slice(64 * t, 64 * t + 64)
            nc.vector.tensor_mul(emb_k[h][e, c], qk_mag[1][r, c], tk_sb[r, t, c])
            nc.vector.tensor_mul(emb_q[h][e, c], qk_mag[0][r, c], tq_sb[r, t, c])

    emb_mul(0, 0)
    v_group_evict(2, psvs[2])
    v_group_evict(3, psvs[3])
    emb_mul(0, 1)
    emb_mul(1, 0)
    emb_mul(1, 1)

    # ---- phase D: attention in two query superblocks of 1024; inside each,
    # (head, key-half) stages. Score tiles double-buffer on scA/scB so the
    # next chunk's matmuls overlap the previous chunk's exp eviction. ----
    av_ps = {}
    exp_tiles = {}

    def scores_chunk(h, kc, qh):
        e = exp_pool.tile([128, QH], dt.bfloat16,
                          name=f"exp{qh}_{h}_{kc}", tag=f"exp{qh}_{kc % 8}")
        exp_tiles[(h, kc, qh)] = e
        sc = ps_pool.tile([128, QH], dt.float32, name=f"sc{qh}_{h}_{kc}",
                          tag=("scA", "scB")[kc % 2])
        for q2 in range(2):
            nc.tensor.matmul(
                sc[:, q2 * 512:(q2 + 1) * 512],
                emb_k[h][:, kc * 128:(kc + 1) * 128],
                emb_q[h][:, qh * QH + q2 * 512:qh * QH + (q2 + 1) * 512],
                start=True, stop=True,
            )
        nc.scalar.activation(e[:], sc[:], AF.Exp, scale=1.0 / math.sqrt(D))

    def av_chunk(h, kc, qh):
        pav = av_ps[(h, qh)]
        e = exp_tiles[(h, kc, qh)]
        for q2 in range(2):
            nc.tensor.matmul(
                pav[0:D + 1, q2 * 512:(q2 + 1) * 512],
                v_sb[:, h, kc, :],
                e[:, q2 * 512:(q2 + 1) * 512],
                start=(kc == 0), stop=(kc == KC - 1),
            )

    def normalize(h, qh):
        # Softmax 1/rowsum off the critical path: rowsum row out first
        # (bf16), DMA-spread over 128 partitions (DVE reciprocal is
        # free-size bound), reciprocal, DMA back, gpsimd broadcast; the
        # attn@v accumulator is evicted bf16 and multiplied in DVE 2x mode.
        pav = av_ps[(h, qh)]
        # partition-aligned rowsum eviction (the verifier requires copy
        # in/out partition ranges to match, so row 64 lands on row 64)
        rsrow = persist.tile([D + 1, QH], dt.bfloat16, name=f"rsr{h}_{qh}",
                             tag=f"rsr{h}")
        with nc.allow_low_precision(reason="softmax rowsum in bf16 is ~0.4% scale noise"):
            nc.vector.tensor_copy(rsrow[D:D + 1, :], pav[D:D + 1, :])
        rs128 = persist.tile([128, QH // 128], dt.bfloat16,
                             name=f"rs128_{h}_{qh}", tag=f"rs128_{h}")
        nc.sync.dma_start(rs128[:], rsrow[D:D + 1, :])
        rr128 = persist.tile([128, QH // 128], dt.bfloat16,
                             name=f"rr128_{h}_{qh}", tag=f"rr128_{h}")
        with nc.allow_low_precision(reason="softmax 1/rowsum in bf16 is ~0.4% scale noise"):
            nc.vector.reciprocal(rr128[:], rs128[:])
        rr = persist.tile([1, QH], dt.bfloat16, name=f"rr{h}_{qh}", tag=f"rr{h}")
        nc.sync.dma_start(rr[:], rr128[:])
        acopy = persist.tile([D, QH], dt.bfloat16,
                             name=f"acopy{h}_{qh}", tag=f"acopy{h}")
        with nc.allow_low_precision(reason="pre-normalize attn@v in bf16, ~0.4%"):
            nc.vector.tensor_copy(acopy[:], pav[0:D, :])
        rsb = persist.tile([64, QH], dt.bfloat16, name=f"rsb{h}_{qh}", tag=f"rsb{h}")
        nc.gpsimd.partition_broadcast(rsb[:], rr[:])
        nc.vector.tensor_mul(outT[64 * h:64 * h + 64, qh * QH:(qh + 1) * QH],
                             acopy[:], rsb[:])

    def oproj(qh, oc, tags=("avA", "avB")):
        # output projection for superblock qh, channel chunk oc; evictions
        # alternate DVE/GpSimd to stay off the exp-pacing ACT engine.
        c = slice(qh * QH, (qh + 1) * QH)
        psy = ps_pool.tile([128, QH], dt.float32, name=f"psy{qh}_{oc}",
                           tag=tags[oc % len(tags)])
        for q2 in range(2):
            nc.tensor.matmul(
                psy[:, q2 * 512:(q2 + 1) * 512],
                wo_sb[:, oc * 128:(oc + 1) * 128],
                outT[:, qh * QH + q2 * 512:qh * QH + (q2 + 1) * 512],
                start=True, stop=True,
            )
        y_sb = out_pool.tile([128, QH], dt.bfloat16, name=f"y{qh}_{oc}", tag="y")
        nc.vector.tensor_copy(y_sb[:], psy[:])
        nc.sync.dma_start(y_ext[oc, :, c], y_sb[:])

    LAG = 1
    for qh in range(2):
        # stage 0: scores/exp (h0, 1st key half); fillers keep PE dense:
        # superblock 1's stage 0 runs superblock 0's output projection on
        # the freed avA/avB banks.
        for j in range(8):
            if qh == 1:
                oproj(0, j)
            scores_chunk(0, j, qh)
        # stage 1: scores/exp (h0, 2nd half) + av (h0, 1st half)
        av_ps[(0, qh)] = ps_pool.tile([128, QH], dt.float32,
                                      name=f"av0_{qh}", tag="avA")
        for j in range(8):
            av_chunk(0, j, qh)
            scores_chunk(0, 8 + j, qh)
        # stage 2: scores/exp (h1, 1st half) + av (h0, 2nd half)
        av_ps[(1, qh)] = ps_pool.tile([128, QH], dt.float32,
                                      name=f"av1_{qh}", tag="avB")
        for j in range(8):
            av_chunk(0, 8 + j, qh)
            scores_chunk(1, j, qh)
        normalize(0, qh)
        # stage 3: scores/exp (h1, 2nd half) + av (h1, 1st half)
        for j in range(8):
            if j >= LAG:
                av_chunk(1, j - LAG, qh)
            scores_chunk(1, 8 + j, qh)
        for kc in range(8 - LAG, KC):
            av_chunk(1, kc, qh)
        if qh == 0:
            normalize(1, qh)

    # ---- phase E: tail. The last normalize (h1, qh1) runs in column
    # halves using ACT ln+exp for 1/rowsum (same act table, ACT is idle
    # here) directly from PSUM; the final output projection chases each
    # half in [128,512] pieces with evictions over DVE/ACT/GpSimd. ----
    pav = av_ps[(1, 1)]
    rsf = persist.tile([D + 1, QH], dt.bfloat16, name="rsf", tag="rsr1")
    with nc.allow_low_precision(reason="softmax rowsum in bf16, ~0.4%"):
        nc.vector.tensor_copy(rsf[D:D + 1, :], pav[D:D + 1, :])
    rs0 = persist.tile([1, QH], dt.bfloat16, name="rs0", tag="rs0")
    nc.sync.dma_start(rs0[:], rsf[D:D + 1, :])
    rrf = persist.tile([1, QH], dt.bfloat16, name="rrf", tag="rrf")
    rsbf = persist.tile([64, QH], dt.bfloat16, name="rsbf", tag="rsbf")
    acf = persist.tile([D, QH], dt.bfloat16, name="acf", tag="acopy1")
    for hf in range(2):
        cl = slice(hf * 512, (hf + 1) * 512)
        lnt = persist.tile([1, 512], dt.float32, name=f"lnt{hf}", tag="lnt")
        nc.scalar.activation(lnt[:], rs0[:, cl], AF.Ln)
        with nc.allow_low_precision(reason="softmax 1/rowsum in bf16, ~0.4%"):
            nc.scalar.activation(rrf[:, cl], lnt[:], AF.Exp,
                                 scale=-1.0)
        nc.gpsimd.partition_broadcast(rsbf[:, cl], rrf[:, cl])
        with nc.allow_low_precision(reason="pre-normalize attn@v in bf16, ~0.4%"):
            nc.vector.tensor_copy(acf[:, cl], pav[0:D, cl])
        nc.vector.tensor_mul(outT[64:128, QH + hf * 512:QH + (hf + 1) * 512],
                             acf[:, cl], rsbf[:, cl])
        tags = ("scA", "scB", "avA", "avB")
        for oc in range(OC):
            psy = ps_pool.tile([128, 512], dt.float32, name=f"psyF{hf}_{oc}",
                               tag=tags[oc % 4])
            nc.tensor.matmul(
                psy[:],
                wo_sb[:, oc * 128:(oc + 1) * 128],
                outT[:, QH + hf * 512:QH + (hf + 1) * 512],
                start=True, stop=True,
            )
            y_sb = out_pool.tile([128, 512], dt.bfloat16,
                                 name=f"yF{hf}_{oc}", tag="y")
            if oc % 2 == 0:
                nc.vector.tensor_copy(y_sb[:], psy[:])
            else:
                nc.scalar.activation(y_sb[:], psy[:], AF.Copy)
            nc.sync.dma_start(
                y_ext[oc, :, QH + hf * 512:QH + (hf + 1) * 512], y_sb[:])


def _build():
    _patch_act_tables()
    nc = bacc.Bacc()
    dt = mybir.dt

    ext = (
        nc.declare_dram_parameter("xt", [128, KI, S], dt.bfloat16, isOutput=False),
        nc.declare_dram_parameter("wqk", [128, 2, KI, ED], dt.bfloat16, isOutput=False),
        nc.declare_dram_parameter("wv", [128, KI, ED], dt.bfloat16, isOutput=False),
        nc.declare_dram_parameter("tq", [128, 2, S], dt.bfloat16, isOutput=False),
        nc.declare_dram_parameter("btr", [128, 2], dt.float32, isOutput=False),
        nc.declare_dram_parameter("woT", [128, DIM], dt.bfloat16, isOutput=False),
        nc.declare_dram_parameter("yT", [OC, 128, S], dt.bfloat16, isOutput=True),
    )

    with tile.TileContext(nc) as tc:
        with tc.tile_pool(name="persist", bufs=1) as persist, \
             tc.tile_pool(name="ps", bufs=1, space="PSUM") as ps_pool, \
             tc.tile_pool(name="out", bufs=4) as out_pool, \
             tc.tile_pool(name="xtp", bufs=1) as xt_pool, \
             tc.tile_pool(name="expp", bufs=2) as exp_pool:
            _build_body(nc, tc, persist, ps_pool, out_pool, xt_pool, exp_pool, ext)
    nc.compile()
    return nc


def _get_nc():
    global _compiled_nc
    if _compiled_nc is None:
        _compiled_nc = _build()
    return _compiled_nc


def _prep_inputs(x, wq, wk, wv, wo, pope_bias):
    """Host-side sharding + layout prep. Returns in_maps for the 8 cores."""
    x2 = np.ascontiguousarray(x.reshape(S, DIM).astype(np.float32))

    # trig tables (f64 phases for accuracy); heads share cos/sin(freqs)
    inv = 10000.0 ** (-(np.arange(D, dtype=np.float64) / D))
    pos = np.arange(S, dtype=np.float64)
    freqs = pos[:, None] * inv[None, :]                       # [S, D]
    bias = np.clip(pope_bias.astype(np.float64), -2 * math.pi, 0.0)  # [H, D]

    tq = np.empty((128, 2, S), BF16)
    tq[0:64, 0] = np.cos(freqs).T
    tq[64:128, 0] = tq[0:64, 0]
    tq[0:64, 1] = np.sin(freqs).T
    tq[64:128, 1] = tq[0:64, 1]

    # xt[q, ki, s] = x[s, ki*128+q]
    xt = np.ascontiguousarray(
        x2.T.reshape(KI, 128, S).transpose(1, 0, 2)).astype(BF16)

    in_maps = []
    for c in range(NCORES):
        hs = slice(c * HPC * D, (c + 1) * HPC * D)            # head-channel slice
        wqk = np.empty((128, 2, KI, ED), BF16)
        for p, wm in enumerate((wq, wk)):
            wt = np.ascontiguousarray(wm[hs, :].astype(np.float32).T)  # [DIM, ED]
            wqk[:, p] = wt.reshape(KI, 128, ED).transpose(1, 0, 2)
        wvt = np.ascontiguousarray(wv[hs, :].astype(np.float32).T)
        wv_m = np.ascontiguousarray(
            wvt.reshape(KI, 128, ED).transpose(1, 0, 2)).astype(BF16)

        # per-head bias trig for on-chip angle addition, head-major rows:
        # partition 64h+d holds (cos(bias[h,d]), sin(bias[h,d]))
        btr = np.empty((128, 2), np.float32)
        for h in range(HPC):
            b = bias[c * HPC + h]                              # [D]
            btr[64 * h:64 * h + 64, 0] = np.cos(b)
            btr[64 * h:64 * h + 64, 1] = np.sin(b)

        woT = np.ascontiguousarray(wo[:, hs].astype(np.float32).T).astype(BF16)

        in_maps.append({
            "xt": xt, "wqk": wqk, "wv": wv_m, "tq": tq, "btr": btr,
            "woT": woT,
        })
    return in_maps


def kernel(x, wq, wk, wv, wo, pope_bias):
    nc = _get_nc()
    in_maps = _prep_inputs(np.asarray(x), np.asarray(wq), np.asarray(wk),
                           np.asarray(wv), np.asarray(wo), np.asarray(pope_bias))
    res = run_bass_kernel_spmd(nc, in_maps, list(range(NCORES)))
    y = np.zeros((DIM, S), np.float32)
    for c in range(NCORES):
        y += res.results[c]["yT"].reshape(DIM, S).astype(np.float32)
    return np.ascontiguousarray(y.T).reshape(1, S, DIM)


if __name__ == "__main__":
    rng = np.random.default_rng(0)
    out = kernel(
        x=rng.standard_normal((1, S, DIM)).astype(np.float32),
        wq=rng.standard_normal((DIM, DIM)).astype(np.float32) / 32,
        wk=rng.standard_normal((DIM, DIM)).astype(np.float32) / 32,
        wv=rng.standard_normal((DIM, DIM)).astype(np.float32) / 32,
        wo=rng.standard_normal((DIM, DIM)).astype(np.float32) / 32,
        pope_bias=-rng.random((H, D), np.float32) * 3.0,
    )
    print("out", out.shape, out.dtype, np.abs(out).mean())


# revision 28
# speedup vs baseline: 1.0215x; 1.0215x over previous
"""PoPE attention kernel for Trainium2, sharded over 8 NeuronCores by heads.

Problem: B=1, S=2048, DIM=1024, H=16 heads, D=64.
  q/k/v = x @ w{q,k,v}^T ; PoPE embed (softplus magnitude x cos/sin phase);
  scores = q_emb @ k_emb^T / sqrt(D); softmax; out = attn @ v; y = out @ wo^T.

Sharding: 2 heads per core. Each core computes its heads' projections,
attention, and a partial output projection (its 128 channels of wo);
host sums the 8 partial y's (f32) - no on-chip collectives.

Key scheduling facts (from ntff traces):
  - ACT paces attention at ~1.2us per [128,1024] exp chunk (64 chunks).
  - PE streams at 0.417 ns/col once warm; total ~84us of matmul columns.
  - Input DMA is ~350 GB/s per core; every input byte delays the start.
Optimizations vs the first working version:
  - Activation tables pinned to natural_log_exp_and_others (exp+ln+copy in
    one set) by masking exp/ln out of the smaller sets before bacc's
    table-load pass: removes 2x 1.5us ACT_TABLE_LOAD switches.
  - trig tables: only cos/sin(freqs) [64,2,S] is DMA'd (heads share it);
    the per-head key phases cos/sin(freqs+bias) are built on DVE via angle
    addition from tiny per-head bias cos/sin - saves 1.5 MB of input DMA.
  - v-projection PSUM evictions go to GpSimd so DVE can start the embed
    multiplies (which gate the first scores matmul) immediately after
    softplus.
  - softmax 1/rowsum: rowsum row is evicted first (bf16), spread over 128
    partitions by DMA, reciprocal on DVE, DMA back, gpsimd broadcast.
    The attn@v accumulator is evicted in bf16 and the normalize multiply
    runs in DVE 2x mode.
  - tail: the last (head, superblock) normalize uses ACT ln+exp (same
    table, ACT is idle in the tail) directly from PSUM instead of the DMA
    round-trip, split in column halves, and the final output projection
    chases it in [128,512] pieces with evictions spread over DVE/ACT/GpSimd.
"""
import math

import numpy as np
import ml_dtypes

import concourse.bacc as bacc
import concourse.mybir as mybir
from concourse import tile
from concourse.bass_utils import run_bass_kernel_spmd

BF16 = ml_dtypes.bfloat16
S, DIM, H, D = 2048, 1024, 16, 64
NCORES = 8
HPC = H // NCORES          # heads per core = 2
ED = 2 * D                 # embedding width per head = 128
KI = DIM // 128            # contraction chunks for projections = 8
KC = S // 128              # key-token chunks = 16
QC = S // 512              # query free-dim chunks of 512 = 4
OC = DIM // 128            # output-channel chunks = 8
QH = 1024                  # query superblock width

_compiled_nc = None
_act_tables_patched = False


def _patch_act_tables():
    """Make bacc's activation-table pass pick natural_log_exp_and_others for
    both Exp and Ln (they otherwise land in two different sets and every
    Exp<->Ln transition costs a ~1.5us ACT_TABLE_LOAD). Masking exp/ln out
    of the smaller sets preserves set indices (act_func_set_id is
    positional) while forcing the combined set."""
    global _act_tables_patched
    if _act_tables_patched:
        return
    _act_tables_patched = True
    orig = bacc.get_activation_tables

    def patched(arch):
        tabs = dict(orig(arch))
        AF = mybir.ActivationFunctionType
        combined = None
        for name, fns in tabs.items():
            if AF.Exp in fns and AF.Ln in fns:
                combined = name
                break
        if combined is None:
            return tabs
        for name, fns in tabs.items():
            if name != combined:
                tabs[name] = fns - {AF.Exp, AF.Ln}
        return tabs

    bacc.get_activation_tables = patched


def _build_body(nc, tc, persist, ps_pool, out_pool, xt_pool, exp_pool, ext):
    dt = mybir.dt
    AF = mybir.ActivationFunctionType
    ALU = mybir.AluOpType
    xt_ext, wqk_ext, wv_ext, tq_ext, btr_ext, wo_ext, y_ext = ext

    # ---- HAM warmup: dummy matmuls on junk data while the input DMAs run,
    # so the PE clock-gate reaches 2.4 GHz before the real matmuls start ----
    warm_sb = persist.tile([128, 512], dt.bfloat16)
    nc.gpsimd.memset(warm_sb[:], 0.0)
    warm_ps = ps_pool.tile([128, 512], dt.float32, name="warm_ps", tag="scA")
    for i in range(16):
        nc.tensor.matmul(warm_ps[:], warm_sb[:, 0:128], warm_sb[:],
                         start=(i == 0), stop=(i == 15))
    # dummy exp: pulls the 1.3us ACT_TABLE_LOAD to t~8us (ACT idle) instead
    # of right before the first softplus exp on the critical path
    warm_act = persist.tile([1, 8], dt.bfloat16, name="warm_act", tag="wact")
    nc.scalar.activation(warm_act[:], warm_sb[0:1, 0:8], AF.Exp)

    # ---- phase A: input DMAs (wqk and xt first so matmuls start early) ----
    wqk_sb = persist.tile([128, 2, KI, ED], dt.bfloat16)
    nc.sync.dma_start(wqk_sb[:], wqk_ext[:])
    xt = xt_pool.tile([128, KI, S], dt.bfloat16)
    for ki in range(KI):
        nc.sync.dma_start(xt[:, ki, :], xt_ext[:, ki, :])
    tq_sb = persist.tile([128, 2, S], dt.bfloat16)
    nc.sync.dma_start(tq_sb[:], tq_ext[:])
    btr_sb = persist.tile([128, 2], dt.float32)
    nc.sync.dma_start(btr_sb[:], btr_ext[:])
    wv_sb = persist.tile([128, KI, ED], dt.bfloat16)
    nc.sync.dma_start(wv_sb[:], wv_ext[:])
    wo_sb = persist.tile([128, DIM], dt.bfloat16)
    nc.sync.dma_start(wo_sb[:], wo_ext[:])

    # key-phase trig tables built on DVE (idle during the input DMA):
    # ck = cq*cos(b) - sq*sin(b); sk = sq*cos(b) + cq*sin(b). btr is
    # head-major per partition so both heads go in one full-width op.
    tk_sb = persist.tile([128, 2, S], dt.bfloat16)
    # scratch shares the softplus tmp tag (trig_k finishes before softplus)
    tk_tmp = xt_pool.tile([128, S], dt.bfloat16, name="tktmp", tag="sp")
    cq, sq = tq_sb[:, 0, :], tq_sb[:, 1, :]
    cb = btr_sb[:, 0:1]
    sb = btr_sb[:, 1:2]
    nc.vector.tensor_scalar_mul(tk_tmp[:], sq, sb)
    nc.vector.scalar_tensor_tensor(
        tk_sb[:, 0, :], cq, cb, tk_tmp[:], ALU.mult, ALU.subtract)
    nc.vector.tensor_scalar_mul(tk_tmp[:], cq, sb)
    nc.vector.scalar_tensor_tensor(
        tk_sb[:, 1, :], sq, cb, tk_tmp[:], ALU.mult, ALU.add)

    # v with a ones column appended per (head, key chunk); the softmax
    # rowsum lands on PSUM partition 64 (32-aligned, which the AP hardware
    # requires; ACT ops only work at partition base 0)
    v_sb = persist.tile([128, HPC, KC, D + 1], dt.bfloat16)
    nc.gpsimd.memset(v_sb[:, 0, :, D], 1.0)
    nc.gpsimd.memset(v_sb[:, 1, :, D], 1.0)
    emb_q = [persist.tile([128, S], dt.bfloat16, name=f"embq{h}", tag=f"embq{h}")
             for h in range(HPC)]
    emb_k = [persist.tile([128, S], dt.bfloat16, name=f"embk{h}", tag=f"embk{h}")
             for h in range(HPC)]
    outT = persist.tile([128, S], dt.bfloat16)

    # PSUM layout: four 2-bank tags. Scores ping-pong on scA/scB while the
    # two attn@v accumulators sit on avA/avB; the projections and the output
    # projection reuse the same four tags.
    # ---- phase B: q/k projections (ki-outer so MMs start after the first
    # xt chunk lands), clustered softplus ----
    psm = {}
    for p in range(2):  # 0=q, 1=k
        for lo in range(2):
            t = ps_pool.tile([128, QH], dt.float32, name=f"psm{p}_{lo}",
                             tag=("scA", "avA", "scB", "avB")[2 * p + lo])
            psm[(p, lo)] = t
        for ki in range(KI):
            for qc in range(QC):
                nc.tensor.matmul(
                    psm[(p, qc // 2)][:, (qc % 2) * 512:(qc % 2) * 512 + 512],
                    wqk_sb[:, p, ki, :],
                    xt[:, ki, qc * 512:(qc + 1) * 512],
                    start=(ki == 0), stop=(ki == KI - 1),
                )
    # softplus(x) = ln(1 + e^x); exp and ln share one activation table
    # (pinned by _patch_act_tables) so no table switches occur.
    tmp = xt_pool.tile([128, 2, S], dt.float32, name="sp", tag="sp")
    mag = xt_pool.tile([128, 2, S], dt.bfloat16, name="mag", tag="mag")
    qk_mag = [mag[:, 0, :], mag[:, 1, :]]
    for p in range(2):
        for lo in range(2):
            nc.scalar.activation(tmp[:, p, lo * QH:(lo + 1) * QH],
                                 psm[(p, lo)][:], AF.Exp)
    nc.scalar.activation(mag[:, :, 0:QH], tmp[:, :, 0:QH], AF.Ln, bias=1.0)
    nc.scalar.activation(mag[:, :, QH:S], tmp[:, :, QH:S], AF.Ln, bias=1.0)

    # ---- phase C: v projection (token-major directly). Only the scA/scB
    # groups' evictions precede the first embeds on the DVE queue; the
    # avA/avB groups evict after the first scores are unblocked. ----
    def v_group_mm(g, tag):
        psv = ps_pool.tile([128, 4, 128], dt.float32, name=f"psv{g}", tag=tag)
        for sub in range(4):
            t = 4 * g + sub
            for ki in range(KI):
                nc.tensor.matmul(
                    psv[:, sub, :],
                    xt[:, ki, t * 128:(t + 1) * 128],
                    wv_sb[:, ki, :],
                    start=(ki == 0), stop=(ki == KI - 1),
                )
        return psv

    def v_group_evict(g, psv):
        for h in range(HPC):
            for sub in range(4):
                t = 4 * g + sub
                nc.vector.tensor_copy(
                    v_sb[:, h, t, 0:D], psv[:, sub, 64 * h:64 * h + 64])

    psvs = [v_group_mm(g, tag)
            for g, tag in enumerate(("scA", "scB", "avA", "avB"))]
    v_group_evict(0, psvs[0])
    v_group_evict(1, psvs[1])

    # embeds on DVE (bf16 SBUF 2x mode), head 0's low half first so its
    # first score chunks can start sooner
    def emb_mul(lo, h):
        c = slice(lo * QH, (lo + 1) * QH)
        r = slice(64 * h, 64 * h + 64)
        for t in range(2):  # 0=cos part, 1=sin part
            e = slice(64 * t, 64 * t + 64)
            nc.vector.tensor_mul(emb_k[h][e, c], qk_mag[1][r, c], tk_sb[r, t, c])
            nc.vector.tensor_mul(emb_q[h][e, c], qk_mag[0][r, c], tq_sb[r, t, c])

    emb_mul(0, 0)
    v_group_evict(2, psvs[2])
    v_group_evict(3, psvs[3])
    emb_mul(0, 1)
    emb_mul(1, 0)
    emb_mul(1, 1)

    # ---- phase D: attention in two query superblocks of 1024; inside each,
    # (head, key-half) stages. Score tiles double-buffer on scA/scB so the
    # next chunk's matmuls overlap the previous chunk's exp eviction. ----
    av_ps = {}
    exp_tiles = {}

    def scores_chunk(h, kc, qh):
        e = exp_pool.tile([128, QH], dt.bfloat16,
                          name=f"exp{qh}_{h}_{kc}", tag=f"exp{qh}_{kc % 8}")
        exp_tiles[(h, kc, qh)] = e
        sc = ps_pool.tile([128, QH], dt.float32, name=f"sc{qh}_{h}_{kc}",
                          tag=("scA", "scB")[kc % 2])
        for q2 in range(2):
            nc.tensor.matmul(
                sc[:, q2 * 512:(q2 + 1) * 512],
                emb_k[h][:, kc * 128:(kc + 1) * 128],
                emb_q[h][:, qh * QH + q2 * 512:qh * QH + (q2 + 1) * 512],
                start=True, stop=True,
            )
        nc.scalar.activation(e[:], sc[:], AF.Exp, scale=1.0 / math.sqrt(D))

    def av_chunk(h, kc, qh, q2s=(0, 1)):
        pav = av_ps[(h, qh)]
        e = exp_tiles[(h, kc, qh)]
        for q2 in q2s:
            nc.tensor.matmul(
                pav[0:D + 1, q2 * 512:(q2 + 1) * 512],
                v_sb[:, h, kc, :],
                e[:, q2 * 512:(q2 + 1) * 512],
                start=(kc == 0), stop=(kc == KC - 1),
            )

    def normalize(h, qh):
        # Softmax 1/rowsum off the critical path: rowsum row (PSUM
        # partition 0, thanks to the prepended ones column) out first in
        # bf16, DMA-spread over 128 partitions (DVE reciprocal is
        # free-size bound), reciprocal, DMA back, gpsimd broadcast; the
        # attn@v accumulator is evicted bf16 and multiplied in DVE 2x mode.
        pav = av_ps[(h, qh)]
        rsrow = persist.tile([D + 1, QH], dt.bfloat16, name=f"rsr{h}_{qh}",
                             tag=f"rsr{h}")
        with nc.allow_low_precision(reason="softmax rowsum in bf16 is ~0.4% scale noise"):
            nc.vector.tensor_copy(rsrow[D:D + 1, :], pav[D:D + 1, :])
        rs128 = persist.tile([128, QH // 128], dt.bfloat16,
                             name=f"rs128_{h}_{qh}", tag=f"rs128_{h}")
        nc.sync.dma_start(rs128[:], rsrow[D:D + 1, :])
        rr128 = persist.tile([128, QH // 128], dt.bfloat16,
                             name=f"rr128_{h}_{qh}", tag=f"rr128_{h}")
        with nc.allow_low_precision(reason="softmax 1/rowsum in bf16 is ~0.4% scale noise"):
            nc.vector.reciprocal(rr128[:], rs128[:])
        rr = persist.tile([1, QH], dt.bfloat16, name=f"rr{h}_{qh}", tag=f"rr{h}")
        nc.sync.dma_start(rr[:], rr128[:])
        acopy = persist.tile([D, QH], dt.bfloat16,
                             name=f"acopy{h}_{qh}", tag=f"acopy{h}")
        with nc.allow_low_precision(reason="pre-normalize attn@v in bf16, ~0.4%"):
            nc.vector.tensor_copy(acopy[:], pav[0:D, :])
        rsb = persist.tile([D, QH], dt.bfloat16, name=f"rsb{h}_{qh}",
                           tag=f"rsb{h}")
        nc.gpsimd.partition_broadcast(rsb[:], rr[:])
        nc.vector.tensor_mul(outT[64 * h:64 * h + 64, qh * QH:(qh + 1) * QH],
                             acopy[:], rsb[:])

    def oproj(qh, oc, tags=("avA", "avB")):
        # output projection for superblock qh, channel chunk oc; evictions
        # alternate DVE/GpSimd to stay off the exp-pacing ACT engine.
        c = slice(qh * QH, (qh + 1) * QH)
        psy = ps_pool.tile([128, QH], dt.float32, name=f"psy{qh}_{oc}",
                           tag=tags[oc % len(tags)])
        for q2 in range(2):
            nc.tensor.matmul(
                psy[:, q2 * 512:(q2 + 1) * 512],
                wo_sb[:, oc * 128:(oc + 1) * 128],
                outT[:, qh * QH + q2 * 512:qh * QH + (q2 + 1) * 512],
                start=True, stop=True,
            )
        y_sb = out_pool.tile([128, QH], dt.bfloat16, name=f"y{qh}_{oc}", tag="y")
        nc.vector.tensor_copy(y_sb[:], psy[:])
        nc.sync.dma_start(y_ext[oc, :, c], y_sb[:])

    LAG = 1
    for qh in range(2):
        # stage 0: scores/exp (h0, 1st key half); fillers keep PE dense:
        # superblock 1's stage 0 runs superblock 0's output projection on
        # the freed avA/avB banks.
        for j in range(8):
            if qh == 1:
                oproj(0, j)
            scores_chunk(0, j, qh)
        # stage 1: scores/exp (h0, 2nd half) + av (h0, 1st half)
        av_ps[(0, qh)] = ps_pool.tile([128, QH], dt.float32,
                                      name=f"av0_{qh}", tag="avA")
        for j in range(8):
            av_chunk(0, j, qh)
            scores_chunk(0, 8 + j, qh)
        # stage 2: scores/exp (h1, 1st half) + av (h0, 2nd half)
        av_ps[(1, qh)] = ps_pool.tile([128, QH], dt.float32,
                                      name=f"av1_{qh}", tag="avB")
        for j in range(8):
            av_chunk(0, 8 + j, qh)
            scores_chunk(1, j, qh)
        normalize(0, qh)
        if qh == 0:
            # stage 3: scores/exp (h1, 2nd half) + av (h1, 1st half)
            for j in range(8):
                if j >= LAG:
                    av_chunk(1, j - LAG, qh)
                scores_chunk(1, 8 + j, qh)
            for kc in range(8 - LAG, KC):
                av_chunk(1, kc, qh)
            normalize(1, qh)
        else:
            # stage 3 of the last superblock: accumulate attn@v for query
            # column-half 0 ONLY, so its normalize + output projection can
            # overlap column-half 1's accumulation (no PE idle tail).
            for j in range(8):
                if j >= LAG:
                    av_chunk(1, j - LAG, qh, q2s=(0,))
                scores_chunk(1, 8 + j, qh)
            for kc in range(8 - LAG, KC):
                av_chunk(1, kc, qh, q2s=(0,))

    # ---- phase E: tail. The (h1, qh1) normalize runs per column half
    # with ACT ln+exp for 1/rowsum (same act table; ACT is idle here; the
    # rowsum is on partition 0 so no relocating DMA is needed), while the
    # PE runs the other half's attn@v, then the final output projection
    # chases in [128,512] pieces with evictions split DVE/ACT. ----
    pav = av_ps[(1, 1)]
    rsff = persist.tile([D + 1, QH], dt.bfloat16, name="rsff", tag="rsr1")
    rs0 = persist.tile([1, QH], dt.bfloat16, name="rs0", tag="rs0")
    rrf = persist.tile([1, QH], dt.bfloat16, name="rrf", tag="rrf")
    rsbf = persist.tile([D, QH], dt.bfloat16, name="rsbf", tag="rsbf")
    acf = persist.tile([D, QH], dt.bfloat16, name="acf", tag="acopy1")

    def norm_final_half(hf):
        cl = slice(hf * 512, (hf + 1) * 512)
        with nc.allow_low_precision(reason="softmax rowsum in bf16, ~0.4%"):
            nc.vector.tensor_copy(rsff[D:D + 1, cl], pav[D:D + 1, cl])
        nc.sync.dma_start(rs0[:, cl], rsff[D:D + 1, cl])
        lnt = persist.tile([1, 512], dt.float32, name=f"lnt{hf}", tag=f"lnt{hf}")
        nc.scalar.activation(lnt[:], rs0[:, cl], AF.Ln)
        with nc.allow_low_precision(reason="softmax 1/rowsum in bf16, ~0.4%"):
            nc.scalar.activation(rrf[:, cl], lnt[:], AF.Exp, scale=-1.0)
        nc.gpsimd.partition_broadcast(rsbf[:, cl], rrf[:, cl])
        with nc.allow_low_precision(reason="pre-normalize attn@v in bf16, ~0.4%"):
            nc.vector.tensor_copy(acf[:, cl], pav[0:D, cl])
        nc.vector.tensor_mul(outT[64:128, QH + hf * 512:QH + (hf + 1) * 512],
                             acf[:, cl], rsbf[:, cl])

    def oproj_final_half(hf, tags):
        for oc in range(OC):
            psy = ps_pool.tile([128, 512], dt.float32, name=f"psyF{hf}_{oc}",
                               tag=tags[oc % len(tags)])
            nc.tensor.matmul(
                psy[:],
                wo_sb[:, oc * 128:(oc + 1) * 128],
                outT[:, QH + hf * 512:QH + (hf + 1) * 512],
                start=True, stop=True,
            )
            y_sb = out_pool.tile([128, 512], dt.bfloat16,
                                 name=f"yF{hf}_{oc}", tag="y")
            if oc % 2 == 0:
                nc.vector.tensor_copy(y_sb[:], psy[:])
            else:
                nc.scalar.activation(y_sb[:], psy[:], AF.Copy)
            nc.sync.dma_start(
                y_ext[oc, :, QH + hf * 512:QH + (hf + 1) * 512], y_sb[:])

    norm_final_half(0)
    for kc in range(KC):
        av_chunk(1, kc, 1, q2s=(1,))
    # pav half-1 columns still accumulate while half 0 projects out; avB
    # (pav) stays live until norm_final_half(1)'s reads complete.
    oproj_final_half(0, ("scA", "scB", "avA"))
    norm_final_half(1)
    oproj_final_half(1, ("scA", "scB", "avA", "avB"))


def _build():
    _patch_act_tables()
    nc = bacc.Bacc()
    dt = mybir.dt

    ext = (
        nc.declare_dram_parameter("xt", [128, KI, S], dt.bfloat16, isOutput=False),
        nc.declare_dram_parameter("wqk", [128, 2, KI, ED], dt.bfloat16, isOutput=False),
        nc.declare_dram_parameter("wv", [128, KI, ED], dt.bfloat16, isOutput=False),
        nc.declare_dram_parameter("tq", [128, 2, S], dt.bfloat16, isOutput=False),
        nc.declare_dram_parameter("btr", [128, 2], dt.float32, isOutput=False),
        nc.declare_dram_parameter("woT", [128, DIM], dt.bfloat16, isOutput=False),
        nc.declare_dram_parameter("yT", [OC, 128, S], dt.bfloat16, isOutput=True),
    )

    with tile.TileContext(nc) as tc:
        with tc.tile_pool(name="persist", bufs=1) as persist, \
             tc.tile_pool(name="ps", bufs=1, space="PSUM") as ps_pool, \
             tc.tile_pool(name="out", bufs=4) as out_pool, \
             tc.tile_pool(name="xtp", bufs=1) as xt_pool, \
             tc.tile_pool(name="expp", bufs=2) as exp_pool:
            _build_body(nc, tc, persist, ps_pool, out_pool, xt_pool, exp_pool, ext)
    nc.compile()
    return nc


def _get_nc():
    global _compiled_nc
    if _compiled_nc is None:
        _compiled_nc = _build()
    return _compiled_nc


def _prep_inputs(x, wq, wk, wv, wo, pope_bias):
    """Host-side sharding + layout prep. Returns in_maps for the 8 cores."""
    x2 = np.ascontiguousarray(x.reshape(S, DIM).astype(np.float32))

    # trig tables (f64 phases for accuracy); heads share cos/sin(freqs)
    inv = 10000.0 ** (-(np.arange(D, dtype=np.float64) / D))
    pos = np.arange(S, dtype=np.float64)
    freqs = pos[:, None] * inv[None, :]                       # [S, D]
    bias = np.clip(pope_bias.astype(np.float64), -2 * math.pi, 0.0)  # [H, D]

    tq = np.empty((128, 2, S), BF16)
    tq[0:64, 0] = np.cos(freqs).T
    tq[64:128, 0] = tq[0:64, 0]
    tq[0:64, 1] = np.sin(freqs).T
    tq[64:128, 1] = tq[0:64, 1]

    # xt[q, ki, s] = x[s, ki*128+q]
    xt = np.ascontiguousarray(
        x2.T.reshape(KI, 128, S).transpose(1, 0, 2)).astype(BF16)

    in_maps = []
    for c in range(NCORES):
        hs = slice(c * HPC * D, (c + 1) * HPC * D)            # head-channel slice
        wqk = np.empty((128, 2, KI, ED), BF16)
        for p, wm in enumerate((wq, wk)):
            wt = np.ascontiguousarray(wm[hs, :].astype(np.float32).T)  # [DIM, ED]
            wqk[:, p] = wt.reshape(KI, 128, ED).transpose(1, 0, 2)
        wvt = np.ascontiguousarray(wv[hs, :].astype(np.float32).T)
        wv_m = np.ascontiguousarray(
            wvt.reshape(KI, 128, ED).transpose(1, 0, 2)).astype(BF16)

        # per-head bias trig for on-chip angle addition, head-major rows:
        # partition 64h+d holds (cos(bias[h,d]), sin(bias[h,d]))
        btr = np.empty((128, 2), np.float32)
        for h in range(HPC):
            b = bias[c * HPC + h]                              # [D]
            btr[64 * h:64 * h + 64, 0] = np.cos(b)
            btr[64 * h:64 * h + 64, 1] = np.sin(b)

        woT = np.ascontiguousarray(wo[:, hs].astype(np.float32).T).astype(BF16)

        in_maps.append({
            "xt": xt, "wqk": wqk, "wv": wv_m, "tq": tq, "btr": btr,
            "woT": woT,
        })
    return in_maps


def kernel(x, wq, wk, wv, wo, pope_bias):
    nc = _get_nc()
    in_maps = _prep_inputs(np.asarray(x), np.asarray(wq), np.asarray(wk),
                           np.asarray(wv), np.asarray(wo), np.asarray(pope_bias))
    res = run_bass_kernel_spmd(nc, in_maps, list(range(NCORES)))
    y = np.zeros((DIM, S), np.float32)
    for c in range(NCORES):
        y += res.results[c]["yT"].reshape(DIM, S).astype(np.float32)
    return np.ascontiguousarray(y.T).reshape(1, S, DIM)


if __name__ == "__main__":
    rng = np.random.default_rng(0)
    out = kernel(
        x=rng.standard_normal((1, S, DIM)).astype(np.float32),
        wq=rng.standard_normal((DIM, DIM)).astype(np.float32) / 32,
        wk=rng.standard_normal((DIM, DIM)).astype(np.float32) / 32,
        wv=rng.standard_normal((DIM, DIM)).astype(np.float32) / 32,
        wo=rng.standard_normal((DIM, DIM)).astype(np.float32) / 32,
        pope_bias=-rng.random((H, D), np.float32) * 3.0,
    )
    print("out", out.shape, out.dtype, np.abs(out).mean())
